# revision 16
# baseline (speedup 1.0000x reference)
"""GraphUnetNoPool (7-layer GCN U-net, no pooling) on 8 trn2 NeuronCores.

Math: gn = D^-1/2 (g+I) D^-1/2;  layer: h' = relu(gn @ h @ W.T + b)
Rewrite: u = dinv*h;  v = (g+I) @ u;  h' = relu((dinv*v) @ W.T + b)
  =>  per-core row-slab m:  v.T[d, m] = sum_k u[k, d] * A[k, m]  (A symmetric:
      column slab of A == transposed row slab, so lhsT = u natural layout and
      rhs = A[:, slab] streams naturally from DRAM rows).
Sharding: 1D row-parallel. Core c owns rows [c*S, (c+1)*S).

Precision: single-pass bf16 everywhere (A exact in bf16, entries {0,1,2};
u/v/W quantized to bf16; fp32 PSUM accumulate). Measured end-to-end rel_l2
vs the fp32 reference ~1.6e-4, threshold 2e-2.

Comm: each layer ends with an AllGather of u, split into `nag` part-slab
collectives launched as soon as their m-chunks finish post-processing, so
they overlap the remaining compute of the layer; the k-loop of the next
layer consumes part-p-sourced chunks in part order so late collectives are
hidden by matmuls on early parts. u rows are kept in part-major order
everywhere (parts of each rank's slab, rank-major within a part) so the
AllGather output layout matches the k-chunk enumeration on every core
without core-dependent addressing.
"""

import os
import numpy as np
from contextlib import ExitStack

import concourse.bass as bass
import concourse.tile as tile
from concourse import bacc, mybir
from concourse.bass_utils import run_bass_kernel_spmd
from concourse.masks import make_identity

F32 = mybir.dt.float32
BF16 = mybir.dt.bfloat16

N, D, C, L = 8192, 256, 8, 7
S = N // C            # 1024 rows per core
KC = N // 128         # 64 k-chunks
MQ = S // 128         # 8 m-chunks per slab
R_DEFAULT = 48        # resident A chunks in SBUF (rest streamed from DRAM)
NAG_DEFAULT = int(os.environ.get("KNAG", "2"))  # AllGather parts per layer


def build_nc(n=N, d=D, c=C, r=R_DEFAULT, n_layers=L, repeat=1, ablate=None,
             nag=None):
    # ablate: None | "noag" (skip collectives; timing-only build) |
    #         "mm1" (additionally skip mm2/transpose/u-prep; timing-only) |
    #         "agonly" (collectives + trivial us writes only; timing-only)
    if nag is None:
        nag = NAG_DEFAULT
    s = n // c
    kc = n // 128
    mq = s // 128
    r = min(r, kc)
    nmh = 2                 # m-halves (psum-bank sized)
    mw = s // nmh           # moving width (512)
    dh_n = d // 128         # d chunks (2 for d=256)
    hq = mq // nmh          # m-chunks per half (4)
    cpp = mq // nag         # m/k-chunks per AG part per rank
    kpp = kc // nag         # k-chunks per AG part (all ranks)
    assert d % 128 == 0 and s % 128 == 0 and n % 128 == 0 and mq % nag == 0

    nc = bacc.Bacc("TRN2", target_bir_lowering=False, debug=False, num_devices=c)

    a_dram = nc.dram_tensor("a_slab", [n, s], F32, kind="ExternalInput")
    u0_dram = nc.dram_tensor("u0", [n, d], BF16, kind="ExternalInput")
    h0s_dram = nc.dram_tensor("h0_slab", [s, d], F32, kind="ExternalInput")
    dslab_dram = nc.dram_tensor("dinv_slab", [128, mq], F32, kind="ExternalInput")
    dbc_dram = nc.dram_tensor("dinv_bcast", [128, s], F32, kind="ExternalInput")
    wt_dram = nc.dram_tensor("wt", [n_layers, d, d], BF16, kind="ExternalInput")
    bias_dram = nc.dram_tensor("bias_t", [128, 2 * n_layers], F32, kind="ExternalInput")
    out_dram = nc.dram_tensor("out", [4, s, d], F32, kind="ExternalOutput")

    UG = max(1, kpp // 8)   # u-load dma groups per part (~8 chunks each)
    kg = kpp // UG

    # global k-chunk index for (part, idx): rank = idx//cpp, j = idx%cpp
    def gchunk(p, idx):
        return (idx // cpp) * mq + p * cpp + (idx % cpp)

    with ExitStack() as ctx:
        tc = ctx.enter_context(tile.TileContext(nc))
        dram = ctx.enter_context(tc.tile_pool(name="dram", bufs=1, space="DRAM"))
        res = ctx.enter_context(tc.tile_pool(name="res", bufs=1))
        stage = ctx.enter_context(tc.tile_pool(name="stage", bufs=2))
        astream = ctx.enter_context(tc.tile_pool(name="astream", bufs=3))
        wtp = ctx.enter_context(tc.tile_pool(name="wtp", bufs=2))
        work = ctx.enter_context(tc.tile_pool(name="work", bufs=2))
        slabp = ctx.enter_context(tc.tile_pool(name="slabp", bufs=2))
        pmm1 = ctx.enter_context(tc.tile_pool(name="pmm1", bufs=4, space="PSUM"))
        post = ctx.enter_context(tc.tile_pool(name="post", bufs=4, space="PSUM"))

        # ---- persistent DRAM scratch ----
        # double-buffered across layers so the input write never waits on the
        # previous layer's collective read
        ag_in = [
            [dram.tile([cpp * 128, d], BF16, name=f"ag_in{b}_{p}") for p in range(nag)]
            for b in range(2)
        ]
        ag_outs = {}
        for i in range((n_layers - 1) * repeat):
            for p in range(nag):
                ag_outs[(i, p)] = dram.tile(
                    [n // nag, d], BF16, name=f"ag_out{i}_{p}", tag=f"ag_out{i}_{p}",
                    addr_space="Shared",
                )
        skip_dram = dram.tile([3, s, d], BF16, name="skip_dram")
        n_spill = kc - r
        if n_spill:
            a_spill = dram.tile([n_spill * 128, s], BF16, name="a_spill")
        ag_dump = {}
        if ablate == "agindep":
            for i in range((n_layers - 1) * repeat):
                for p in range(nag):
                    ag_dump[(i, p)] = dram.tile(
                        [n // nag, d], BF16, name=f"ag_dump{i}_{p}",
                        tag=f"ag_dump{i}_{p}", addr_space="Shared",
                    )

        # ---- persistent SBUF ----
        a_sb = res.tile([128, max(r, 1), s], BF16, name="a_sb")
        u_p = [res.tile([128, kpp, d], BF16, name=f"u_p{p}") for p in range(nag)]
        dinv_sb = res.tile([128, mq], F32, name="dinv_sb")
        dinv_bc = res.tile([128, s], F32, name="dinv_bc")
        bias_sb = res.tile([128, 2 * n_layers], F32, name="bias_sb")
        ident = res.tile([128, 128], F32, name="ident")

        make_identity(nc, ident)
        nc.sync.dma_start(out=dinv_sb, in_=dslab_dram[:, :])
        nc.sync.dma_start(out=dinv_bc, in_=dbc_dram[:, :])
        nc.sync.dma_start(out=bias_sb, in_=bias_dram[:, :])

        # ---- startup: load A column-slab, cast to bf16 (resident + spill) ----
        for k in range(kc):
            st = stage.tile([128, s], F32, name="st", tag="stage")
            nc.sync.dma_start(out=st, in_=a_dram[k * 128 : (k + 1) * 128, :])
            if k < r:
                nc.vector.tensor_copy(a_sb[:, k, :], st)
            else:
                sb16 = stage.tile([128, s], BF16, name="sb16", tag="spill16")
                nc.vector.tensor_copy(sb16, st)
                nc.sync.dma_start(
                    out=a_spill[(k - r) * 128 : (k - r + 1) * 128, :], in_=sb16
                )

        if ablate == "agindep":
            # dummy AG inputs written once; collectives data-independent of
            # compute (outputs go to ag_dump, never read)
            ud = res.tile([128, cpp, d], BF16, name="ud")
            for j in range(cpp):
                nc.scalar.copy(ud[:, j, :], a_sb[:, 0, 0:d])
            for p in range(nag):
                nc.sync.dma_start(
                    out=ag_in[0][p].rearrange("(mm pp) dd -> pp mm dd", pp=128),
                    in_=ud,
                )

        relu = mybir.ActivationFunctionType.Relu
        skip_slot = {4: 2, 5: 1, 6: 0}  # up-layer l uses skip h_{...} slot

        for rep_l in range(n_layers * repeat):
            rep, l = divmod(rep_l, n_layers)
            # ---- Phase A: load U parts (layer 0: host-packed u0; else AG) ----
            if ablate != "agonly":
                for p in range(nag):
                    if l == 0:
                        src = u0_dram[p * (n // nag) : (p + 1) * (n // nag), :]
                    else:
                        src = ag_outs[((rep * (n_layers - 1) + l - 1), p)]
                    src3 = src.rearrange("(g k pp) dd -> g pp k dd", pp=128, g=UG)
                    for g in range(UG):
                        # ACT-engine DMA queue: these are the only DMAs that
                        # wait on collectives — keep them off the SP queue so
                        # they can't head-of-line-block spill streams or the
                        # next collective's input copy
                        nc.scalar.dma_start(
                            out=u_p[p][:, g * kg : (g + 1) * kg, :], in_=src3[g]
                        )

            # per-layer weight prefetch (bf16)
            wt_t = wtp.tile([128, dh_n, d], BF16, name="wt_t", tag="wt")
            nc.sync.dma_start(
                out=wt_t, in_=wt_dram[l].rearrange("(kc p) o -> p kc o", p=128)
            )

            # skip-connection preload for NEXT layer's input (scaled by dinv)
            nl = l + 1
            skip16 = None
            if nl in skip_slot and nl < n_layers and ablate != "agonly":
                skip16 = slabp.tile([128, mq, d], BF16, name="skip16", tag="skip")
                nc.sync.dma_start(
                    out=skip16,
                    in_=skip_dram[skip_slot[nl]].rearrange(
                        "(m p) d2 -> p m d2", p=128
                    ),
                )
                for m in range(mq):
                    nc.vector.tensor_scalar(
                        out=skip16[:, m, :],
                        in0=skip16[:, m, :],
                        scalar1=dinv_sb[:, m : m + 1],
                        scalar2=None,
                        op0=mybir.AluOpType.mult,
                    )

            is_out = l >= n_layers - 3  # layers 4,5,6 emit outputs 0,1,2
            save_skip = l <= 2
            h_nat = None
            h_skip = None
            if is_out:
                h_nat = slabp.tile([128, mq, d], F32, name="h_nat", tag="hnat", bufs=1)
            if save_skip:
                h_skip = slabp.tile([128, mq, d], BF16, name="h_skip", tag="hskip", bufs=1)
            if l == n_layers - 1:
                h0s = slabp.tile([128, mq, d], F32, name="h0s", tag="h0s", bufs=1)
                nc.sync.dma_start(
                    out=h0s, in_=h0s_dram[:, :].rearrange("(m p) d2 -> p m d2", p=128)
                )
                out3 = slabp.tile([128, mq, d], F32, name="out3", tag="out3", bufs=1)
            us_hi = None
            if l < n_layers - 1:
                us_hi = slabp.tile([128, mq, d], BF16, name="us_hi", tag="us_hi")

            v16 = [work.tile([128, s], BF16, name="v16", tag="vsb") for _ in range(dh_n)]
            hT = [work.tile([128, s], F32, name="hT", tag="hT") for _ in range(dh_n)]

            def launch_ag(m):
                # AllGather part p once its m-chunks are finalized
                if us_hi is None or (m + 1) % cpp != 0:
                    return
                p = m // cpp
                agi = ag_in[rep_l % 2][p]
                agv = agi.rearrange("(mm pp) dd -> pp mm dd", pp=128)
                nc.sync.dma_start(
                    out=agv, in_=us_hi[:, p * cpp : (p + 1) * cpp, :]
                )
                nc.gpsimd.collective_compute(
                    "AllGather",
                    mybir.AluOpType.bypass,
                    replica_groups=[list(range(c))],
                    ins=[agi.opt()],
                    outs=[ag_outs[(rep * (n_layers - 1) + l, p)].opt()],
                )

            # ---- per m-half: mm1, mm2, relu, transpose, u-prep, AG ----
            for mh in range(nmh):
                msl = slice(mh * mw, (mh + 1) * mw)
                if ablate == "agonly":
                    for m in range(mh * hq, (mh + 1) * hq):
                        if us_hi is not None:
                            nc.scalar.copy(us_hi[:, m, :], a_sb[:, 0, 0:d])
                        launch_ag(m)
                    continue
                psv = [pmm1.tile([128, mw], F32, name="psv", tag="pmm1") for _ in range(dh_n)]
                # mm1: accumulate over k, AG-part-0-sourced chunks first
                for p in range(nag):
                    for idx in range(kpp):
                        g = gchunk(p, idx)
                        if g < r:
                            rhs = a_sb[:, g, msl]
                        else:
                            ast = astream.tile([128, mw], BF16, name="ast", tag="astream")
                            nc.sync.dma_start(
                                out=ast,
                                in_=a_spill[(g - r) * 128 : (g - r + 1) * 128, msl],
                            )
                            rhs = ast
                        lt = u_p[p][:, idx, :]
                        first = p == 0 and idx == 0
                        last = p == nag - 1 and idx == kpp - 1
                        for dh in range(dh_n):
                            nc.tensor.matmul(
                                psv[dh],
                                lt[:, dh * 128 : (dh + 1) * 128],
                                rhs,
                                start=first,
                                stop=last,
                            )
                # v.T to SBUF as bf16
                for dh in range(dh_n):
                    nc.vector.tensor_copy(v16[dh][:, msl], psv[dh])
                if ablate == "mm1":
                    continue
                # mm2 (bf16)
                pso = [post.tile([128, mw], F32, name="pso", tag="post") for _ in range(dh_n)]
                for dho in range(dh_n):
                    for kin in range(dh_n):
                        nc.tensor.matmul(
                            pso[dho],
                            wt_t[:, kin, dho * 128 : (dho + 1) * 128],
                            v16[kin][:, msl],
                            start=(kin == 0),
                            stop=(kin == dh_n - 1),
                        )
                for dho in range(dh_n):
                    nc.vector.tensor_mul(hT[dho][:, msl], pso[dho], dinv_bc[:, msl])
                    nc.scalar.activation(
                        hT[dho][:, msl],
                        hT[dho][:, msl],
                        relu,
                        bias=bias_sb[:, 2 * l + dho : 2 * l + dho + 1],
                    )
                for m in range(mh * hq, (mh + 1) * hq):
                    tp = post.tile([128, d], F32, name="tp", tag="post")
                    for dh in range(dh_n):
                        nc.tensor.transpose(
                            tp[:, dh * 128 : (dh + 1) * 128],
                            hT[dh][:, m * 128 : (m + 1) * 128],
                            ident,
                        )
                    if us_hi is not None:
                        dv = dinv_sb[:, m : m + 1]
                        if skip16 is not None:
                            nc.vector.scalar_tensor_tensor(
                                out=us_hi[:, m, :],
                                in0=tp,
                                scalar=dv,
                                in1=skip16[:, m, :],
                                op0=mybir.AluOpType.mult,
                                op1=mybir.AluOpType.add,
                            )
                        else:
                            nc.vector.tensor_scalar(
                                out=us_hi[:, m, :],
                                in0=tp,
                                scalar1=dv,
                                scalar2=None,
                                op0=mybir.AluOpType.mult,
                            )
                    if h_skip is not None:
                        nc.scalar.copy(h_skip[:, m, :], tp)
                    if h_nat is not None:
                        nc.scalar.copy(h_nat[:, m, :], tp)
                    if l == n_layers - 1:
                        nc.vector.tensor_add(out3[:, m, :], tp, h0s[:, m, :])
                    if ablate is None:
                        launch_ag(m)
                    elif (
                        ablate == "agindep"
                        and us_hi is not None
                        and (m + 1) % cpp == 0
                    ):
                        p = m // cpp
                        nc.gpsimd.collective_compute(
                            "AllGather",
                            mybir.AluOpType.bypass,
                            replica_groups=[list(range(c))],
                            ins=[ag_in[0][p].opt()],
                            outs=[ag_dump[(rep * (n_layers - 1) + l, p)].opt()],
                        )

            # ---- Phase F: slab-sized DMAs out ----
            if ablate in ("mm1", "agonly"):
                continue
            if save_skip:
                nc.sync.dma_start(
                    out=skip_dram[l].rearrange("(m p) d2 -> p m d2", p=128),
                    in_=h_skip,
                )
            if is_out:
                nc.sync.dma_start(
                    out=out_dram[l - (n_layers - 3)].rearrange(
                        "(m p) d2 -> p m d2", p=128
                    ),
                    in_=h_nat,
                )
            if l == n_layers - 1:
                nc.sync.dma_start(
                    out=out_dram[3].rearrange("(m p) d2 -> p m d2", p=128), in_=out3
                )

    nc.compile()
    return nc


def prep_inputs(g, h, W_down, b_down, W_bottom, b_bottom, W_up, b_up, c=C,
                nag=None):
    """Host-side sharding + layout prep. Returns per-core input maps."""
    if nag is None:
        nag = NAG_DEFAULT
    n = g.shape[0]
    s = n // c
    d = h.shape[1]
    g = np.asarray(g, np.float32)
    h = np.asarray(h, np.float32)
    deg = g.sum(axis=1) + 1.0
    dinv = (1.0 / np.sqrt(deg)).astype(np.float32)

    u0 = (h * dinv[:, None]).astype(np.float32)
    # part-major permutation: [part, rank, rows-within-part]
    u0_perm = np.ascontiguousarray(
        u0.reshape(c, nag, s // nag, d).transpose(1, 0, 2, 3).reshape(n, d)
    )
    u0_packed = np.asarray(u0_perm.astype(ml_bf16))

    Ws = [W_down[0], W_down[1], W_down[2], W_bottom, W_up[0], W_up[1], W_up[2]]
    bs = [b_down[0], b_down[1], b_down[2], b_bottom, b_up[0], b_up[1], b_up[2]]
    wt = np.stack(
        [np.ascontiguousarray(np.asarray(W, np.float32).T) for W in Ws]
    ).astype(ml_bf16)
    nl = len(Ws)
    bias_t = np.zeros((128, 2 * nl), np.float32)
    for li, b in enumerate(bs):
        b = np.asarray(b, np.float32)
        for dh in range(d // 128):
            bias_t[:, 2 * li + dh] = b[dh * 128 : (dh + 1) * 128]

    in_maps = []
    for ci in range(c):
        sl = slice(ci * s, (ci + 1) * s)
        a_slab = np.ascontiguousarray(g[:, sl])
        idx = np.arange(s)
        a_slab[ci * s + idx, idx] += 1.0  # fold self-loops into the slab
        dinv_slab = dinv[sl].reshape(s // 128, 128).T.copy()  # [128, mq]
        dinv_bcast = np.broadcast_to(dinv[sl][None, :], (128, s)).copy()
        in_maps.append(
            dict(
                a_slab=a_slab,
                u0=u0_packed,
                h0_slab=np.ascontiguousarray(h[sl]),
                dinv_slab=dinv_slab,
                dinv_bcast=dinv_bcast,
                wt=np.asarray(wt),
                bias_t=bias_t,
            )
        )
    return in_maps


try:
    import ml_dtypes

    ml_bf16 = ml_dtypes.bfloat16
except ImportError:  # pragma: no cover
    import jax.numpy as jnp

    ml_bf16 = jnp.bfloat16

_NC_CACHE = {}


def kernel(g, h, W_down, b_down, W_bottom, b_bottom, W_up, b_up):
    key = "full"
    if key not in _NC_CACHE:
        _NC_CACHE[key] = build_nc()
    nc = _NC_CACHE[key]
    in_maps = prep_inputs(g, h, W_down, b_down, W_bottom, b_bottom, W_up, b_up)
    res = run_bass_kernel_spmd(nc, in_maps, list(range(C)))
    outs = [np.asarray(r["out"]).reshape(4, S, D) for r in res.results]
    full = np.concatenate(outs, axis=1)  # [4, N, D]
    return full.astype(np.float32)


if __name__ == "__main__":
    import reference

    inputs = reference.setup_inputs()
    inputs = {k: np.asarray(v) for k, v in inputs.items()}
    out = kernel(**inputs)
    exp = np.asarray(reference.reference(**reference.setup_inputs()))
    err = np.abs(out - exp).max() / (np.abs(exp).max() + 1e-30)
    rel = np.linalg.norm(out - exp) / (np.linalg.norm(exp) + 1e-30)
    print("max-scaled err:", err, "rel l2:", rel)


# revision 21
# speedup vs baseline: 1.9739x; 1.9739x over previous
"""GraphUnetNoPool (7-layer GCN U-net, no pooling) on 8 trn2 NeuronCores.

Math: gn = D^-1/2 (g+I) D^-1/2;  layer: h' = relu(gn @ h @ W.T + b)
Rewrite: u = dinv*h;  v = (g+I) @ u;  h' = relu((dinv*v) @ W.T + b)
  =>  per-core row-slab m:  v.T[d, m] = sum_k u[k, d] * A[k, m]  (A symmetric:
      column slab of A == transposed row slab, so lhsT = u natural layout and
      rhs = A[:, slab] streams naturally from DRAM rows).
Sharding: 1D row-parallel. Core c owns rows [c*S, (c+1)*S).

Precision: single-pass bf16 everywhere (A exact in bf16, entries {0,1,2};
u/v/W quantized to bf16; fp32 PSUM accumulate). Measured end-to-end rel_l2
vs the fp32 reference ~1.6e-4, threshold 2e-2.

Comm: each layer ends with an AllGather of u, split into `nag` part-slab
collectives launched as soon as their m-chunks finish post-processing, so
they overlap the remaining compute of the layer; the k-loop of the next
layer consumes part-p-sourced chunks in part order so late collectives are
hidden by matmuls on early parts. u rows are kept in part-major order
everywhere (parts of each rank's slab, rank-major within a part) so the
AllGather output layout matches the k-chunk enumeration on every core
without core-dependent addressing.
"""

import os
import numpy as np
from contextlib import ExitStack

import concourse.bass as bass
import concourse.tile as tile
from concourse import bacc, mybir
from concourse.bass_utils import run_bass_kernel_spmd
from concourse.masks import make_identity

F32 = mybir.dt.float32
BF16 = mybir.dt.bfloat16

N, D, C, L = 8192, 256, 8, 7
S = N // C            # 1024 rows per core
KC = N // 128         # 64 k-chunks
MQ = S // 128         # 8 m-chunks per slab
R_DEFAULT = 48        # resident A chunks in SBUF (rest streamed from DRAM)
NAG_DEFAULT = int(os.environ.get("KNAG", "2"))  # AllGather parts per layer


def build_nc(n=N, d=D, c=C, r=R_DEFAULT, n_layers=L, repeat=1, ablate=None,
             nag=None):
    # ablate: None | "noag" (skip collectives; timing-only build) |
    #         "mm1" (additionally skip mm2/transpose/u-prep; timing-only) |
    #         "agonly" (collectives + trivial us writes only; timing-only)
    if nag is None:
        nag = NAG_DEFAULT
    s = n // c
    kc = n // 128
    mq = s // 128
    r = min(r, kc)
    nmh = 2                 # m-halves (psum-bank sized)
    mw = s // nmh           # moving width (512)
    dh_n = d // 128         # d chunks (2 for d=256)
    hq = mq // nmh          # m-chunks per half (4)
    cpp = mq // nag         # m/k-chunks per AG part per rank
    kpp = kc // nag         # k-chunks per AG part (all ranks)
    assert d % 128 == 0 and s % 128 == 0 and n % 128 == 0 and mq % nag == 0

    nc = bacc.Bacc("TRN2", target_bir_lowering=False, debug=False, num_devices=c)

    a_dram = nc.dram_tensor("a_slab", [n, s], F32, kind="ExternalInput")
    u0_dram = nc.dram_tensor("u0", [n, d], BF16, kind="ExternalInput")
    h0s_dram = nc.dram_tensor("h0_slab", [s, d], F32, kind="ExternalInput")
    dslab_dram = nc.dram_tensor("dinv_slab", [128, mq], F32, kind="ExternalInput")
    dbc_dram = nc.dram_tensor("dinv_bcast", [128, s], F32, kind="ExternalInput")
    wt_dram = nc.dram_tensor("wt", [n_layers, d, d], BF16, kind="ExternalInput")
    bias_dram = nc.dram_tensor("bias_t", [128, 2 * n_layers], F32, kind="ExternalInput")
    out_dram = nc.dram_tensor("out", [4, s, d], F32, kind="ExternalOutput")

    UG = max(1, kpp // 8)   # u-load dma groups per part (~8 chunks each)
    kg = kpp // UG

    # global k-chunk index for (part, idx): rank = idx//cpp, j = idx%cpp
    def gchunk(p, idx):
        return (idx // cpp) * mq + p * cpp + (idx % cpp)

    with ExitStack() as ctx:
        tc = ctx.enter_context(tile.TileContext(nc))
        dram = ctx.enter_context(tc.tile_pool(name="dram", bufs=1, space="DRAM"))
        res = ctx.enter_context(tc.tile_pool(name="res", bufs=1))
        stage = ctx.enter_context(tc.tile_pool(name="stage", bufs=2))
        astream = ctx.enter_context(tc.tile_pool(name="astream", bufs=3))
        wtp = ctx.enter_context(tc.tile_pool(name="wtp", bufs=2))
        work = ctx.enter_context(tc.tile_pool(name="work", bufs=2))
        slabp = ctx.enter_context(tc.tile_pool(name="slabp", bufs=2))
        pmm1 = ctx.enter_context(tc.tile_pool(name="pmm1", bufs=4, space="PSUM"))
        post = ctx.enter_context(tc.tile_pool(name="post", bufs=4, space="PSUM"))

        # ---- persistent DRAM scratch ----
        # double-buffered across layers so the input write never waits on the
        # previous layer's collective read
        ag_in = [
            [dram.tile([cpp * 128, d], BF16, name=f"ag_in{b}_{p}") for p in range(nag)]
            for b in range(2)
        ]
        ag_outs = {}
        for i in range((n_layers - 1) * repeat):
            for p in range(nag):
                ag_outs[(i, p)] = dram.tile(
                    [n // nag, d], BF16, name=f"ag_out{i}_{p}", tag=f"ag_out{i}_{p}",
                    addr_space="Shared",
                )
        skip_dram = dram.tile([3, s, d], BF16, name="skip_dram")
        n_spill = kc - r
        if n_spill:
            a_spill = dram.tile([n_spill * 128, s], BF16, name="a_spill")
        ag_dump = {}
        if ablate in ("agindep", "agout"):
            for i in range((n_layers - 1) * repeat):
                for p in range(nag):
                    ag_dump[(i, p)] = dram.tile(
                        [n // nag, d], BF16, name=f"ag_dump{i}_{p}",
                        tag=f"ag_dump{i}_{p}", addr_space="Shared",
                    )

        # ---- persistent SBUF ----
        a_sb = res.tile([128, max(r, 1), s], BF16, name="a_sb")
        u_p = [res.tile([128, kpp, d], BF16, name=f"u_p{p}") for p in range(nag)]
        dinv_sb = res.tile([128, mq], F32, name="dinv_sb")
        dinv_bc = res.tile([128, s], F32, name="dinv_bc")
        bias_sb = res.tile([128, 2 * n_layers], F32, name="bias_sb")
        ident = res.tile([128, 128], F32, name="ident")

        make_identity(nc, ident)
        nc.sync.dma_start(out=dinv_sb, in_=dslab_dram[:, :])
        nc.sync.dma_start(out=dinv_bc, in_=dbc_dram[:, :])
        nc.sync.dma_start(out=bias_sb, in_=bias_dram[:, :])

        # ---- startup: load A column-slab, cast to bf16 (resident + spill) ----
        for k in range(kc):
            st = stage.tile([128, s], F32, name="st", tag="stage")
            nc.sync.dma_start(out=st, in_=a_dram[k * 128 : (k + 1) * 128, :])
            if k < r:
                nc.vector.tensor_copy(a_sb[:, k, :], st)
            else:
                sb16 = stage.tile([128, s], BF16, name="sb16", tag="spill16")
                nc.vector.tensor_copy(sb16, st)
                nc.sync.dma_start(
                    out=a_spill[(k - r) * 128 : (k - r + 1) * 128, :], in_=sb16
                )

        if ablate in ("agindep", "aginpre"):
            # dummy AG inputs written once (pre-staged)
            ud = res.tile([128, cpp, d], BF16, name="ud")
            for j in range(cpp):
                nc.scalar.copy(ud[:, j, :], a_sb[:, 0, 0:d])
            for p in range(nag):
                nc.sync.dma_start(
                    out=ag_in[0][p].rearrange("(mm pp) dd -> pp mm dd", pp=128),
                    in_=ud,
                )

        relu = mybir.ActivationFunctionType.Relu
        skip_slot = {4: 2, 5: 1, 6: 0}  # up-layer l uses skip h_{...} slot

        for rep_l in range(n_layers * repeat):
            rep, l = divmod(rep_l, n_layers)
            # ---- Phase A: load U parts (layer 0: host-packed u0; else AG) ----
            if ablate != "agonly":
                for p in range(nag):
                    if l == 0:
                        src = u0_dram[p * (n // nag) : (p + 1) * (n // nag), :]
                    else:
                        src = ag_outs[((rep * (n_layers - 1) + l - 1), p)]
                    src3 = src.rearrange("(g k pp) dd -> g pp k dd", pp=128, g=UG)
                    for g in range(UG):
                        # ACT-engine DMA queue: these are the only DMAs that
                        # wait on collectives — keep them off the SP queue so
                        # they can't head-of-line-block spill streams or the
                        # next collective's input copy
                        nc.scalar.dma_start(
                            out=u_p[p][:, g * kg : (g + 1) * kg, :], in_=src3[g]
                        )

            # per-layer weight prefetch (bf16)
            wt_t = wtp.tile([128, dh_n, d], BF16, name="wt_t", tag="wt")
            nc.sync.dma_start(
                out=wt_t, in_=wt_dram[l].rearrange("(kc p) o -> p kc o", p=128)
            )

            # skip-connection preload for NEXT layer's input (scaled by dinv)
            nl = l + 1
            skip16 = None
            if nl in skip_slot and nl < n_layers and ablate != "agonly":
                skip16 = slabp.tile([128, mq, d], BF16, name="skip16", tag="skip")
                nc.sync.dma_start(
                    out=skip16,
                    in_=skip_dram[skip_slot[nl]].rearrange(
                        "(m p) d2 -> p m d2", p=128
                    ),
                )
                for m in range(mq):
                    nc.vector.tensor_scalar(
                        out=skip16[:, m, :],
                        in0=skip16[:, m, :],
                        scalar1=dinv_sb[:, m : m + 1],
                        scalar2=None,
                        op0=mybir.AluOpType.mult,
                    )

            is_out = l >= n_layers - 3  # layers 4,5,6 emit outputs 0,1,2
            save_skip = l <= 2
            h_nat = None
            h_skip = None
            if is_out:
                h_nat = slabp.tile([128, mq, d], F32, name="h_nat", tag="hnat", bufs=1)
            if save_skip:
                h_skip = slabp.tile([128, mq, d], BF16, name="h_skip", tag="hskip", bufs=1)
            if l == n_layers - 1:
                h0s = slabp.tile([128, mq, d], F32, name="h0s", tag="h0s", bufs=1)
                nc.sync.dma_start(
                    out=h0s, in_=h0s_dram[:, :].rearrange("(m p) d2 -> p m d2", p=128)
                )
                out3 = slabp.tile([128, mq, d], F32, name="out3", tag="out3", bufs=1)
            us_hi = None
            if l < n_layers - 1:
                us_hi = slabp.tile([128, mq, d], BF16, name="us_hi", tag="us_hi")

            v16 = [work.tile([128, s], BF16, name="v16", tag="vsb") for _ in range(dh_n)]
            hT = [work.tile([128, s], F32, name="hT", tag="hT") for _ in range(dh_n)]

            def launch_ag(m):
                # AllGather part p once its m-chunks are finalized
                if us_hi is None or (m + 1) % cpp != 0:
                    return
                p = m // cpp
                li = rep * (n_layers - 1) + l
                if ablate == "aginpre":
                    # pre-staged dummy input; real consumed output
                    nc.gpsimd.collective_compute(
                        "AllGather",
                        mybir.AluOpType.bypass,
                        replica_groups=[list(range(c))],
                        ins=[ag_in[0][p].opt()],
                        outs=[ag_outs[(li, p)].opt()],
                    )
                    return
                agi = ag_in[rep_l % 2][p]
                agv = agi.rearrange("(mm pp) dd -> pp mm dd", pp=128)
                # gpsimd-issued input copy: keeps the collective's input write
                # in-stream on the same engine, so the collective's wait is
                # already satisfied when gpsimd reaches it
                nc.gpsimd.dma_start(
                    out=agv, in_=us_hi[:, p * cpp : (p + 1) * cpp, :]
                )
                out_t = (
                    ag_dump[(li, p)] if ablate == "agout" else ag_outs[(li, p)]
                )
                nc.gpsimd.collective_compute(
                    "AllGather",
                    mybir.AluOpType.bypass,
                    replica_groups=[list(range(c))],
                    ins=[agi.opt()],
                    outs=[out_t.opt()],
                )

            # ---- per m-half: mm1, mm2, relu, transpose, u-prep, AG ----
            for mh in range(nmh):
                msl = slice(mh * mw, (mh + 1) * mw)
                if ablate == "agonly":
                    for m in range(mh * hq, (mh + 1) * hq):
                        if us_hi is not None:
                            nc.scalar.copy(us_hi[:, m, :], a_sb[:, 0, 0:d])
                        launch_ag(m)
                    continue
                psv = [pmm1.tile([128, mw], F32, name="psv", tag="pmm1") for _ in range(dh_n)]
                # mm1: accumulate over k, AG-part-0-sourced chunks first
                for p in range(nag):
                    for idx in range(kpp):
                        g = gchunk(p, idx)
                        if g < r:
                            rhs = a_sb[:, g, msl]
                        else:
                            ast = astream.tile([128, mw], BF16, name="ast", tag="astream")
                            nc.sync.dma_start(
                                out=ast,
                                in_=a_spill[(g - r) * 128 : (g - r + 1) * 128, msl],
                            )
                            rhs = ast
                        lt = u_p[p][:, idx, :]
                        first = p == 0 and idx == 0
                        last = p == nag - 1 and idx == kpp - 1
                        for dh in range(dh_n):
                            nc.tensor.matmul(
                                psv[dh],
                                lt[:, dh * 128 : (dh + 1) * 128],
                                rhs,
                                start=first,
                                stop=last,
                            )
                # v.T to SBUF as bf16
                for dh in range(dh_n):
                    nc.vector.tensor_copy(v16[dh][:, msl], psv[dh])
                if ablate == "mm1":
                    continue
                # mm2 (bf16)
                pso = [post.tile([128, mw], F32, name="pso", tag="post") for _ in range(dh_n)]
                for dho in range(dh_n):
                    for kin in range(dh_n):
                        nc.tensor.matmul(
                            pso[dho],
                            wt_t[:, kin, dho * 128 : (dho + 1) * 128],
                            v16[kin][:, msl],
                            start=(kin == 0),
                            stop=(kin == dh_n - 1),
                        )
                for dho in range(dh_n):
                    nc.vector.tensor_mul(hT[dho][:, msl], pso[dho], dinv_bc[:, msl])
                    nc.scalar.activation(
                        hT[dho][:, msl],
                        hT[dho][:, msl],
                        relu,
                        bias=bias_sb[:, 2 * l + dho : 2 * l + dho + 1],
                    )
                for m in range(mh * hq, (mh + 1) * hq):
                    tp = post.tile([128, d], F32, name="tp", tag="post")
                    for dh in range(dh_n):
                        nc.tensor.transpose(
                            tp[:, dh * 128 : (dh + 1) * 128],
                            hT[dh][:, m * 128 : (m + 1) * 128],
                            ident,
                        )
                    if us_hi is not None:
                        dv = dinv_sb[:, m : m + 1]
                        if skip16 is not None:
                            nc.vector.scalar_tensor_tensor(
                                out=us_hi[:, m, :],
                                in0=tp,
                                scalar=dv,
                                in1=skip16[:, m, :],
                                op0=mybir.AluOpType.mult,
                                op1=mybir.AluOpType.add,
                            )
                        else:
                            nc.vector.tensor_scalar(
                                out=us_hi[:, m, :],
                                in0=tp,
                                scalar1=dv,
                                scalar2=None,
                                op0=mybir.AluOpType.mult,
                            )
                    if h_skip is not None:
                        nc.scalar.copy(h_skip[:, m, :], tp)
                    if h_nat is not None:
                        nc.scalar.copy(h_nat[:, m, :], tp)
                    if l == n_layers - 1:
                        nc.vector.tensor_add(out3[:, m, :], tp, h0s[:, m, :])
                    if ablate in (None, "agout", "aginpre"):
                        launch_ag(m)
                    elif (
                        ablate == "agindep"
                        and us_hi is not None
                        and (m + 1) % cpp == 0
                    ):
                        p = m // cpp
                        nc.gpsimd.collective_compute(
                            "AllGather",
                            mybir.AluOpType.bypass,
                            replica_groups=[list(range(c))],
                            ins=[ag_in[0][p].opt()],
                            outs=[ag_dump[(rep * (n_layers - 1) + l, p)].opt()],
                        )

            # ---- Phase F: slab-sized DMAs out ----
            if ablate in ("mm1", "agonly"):
                continue
            if save_skip:
                nc.sync.dma_start(
                    out=skip_dram[l].rearrange("(m p) d2 -> p m d2", p=128),
                    in_=h_skip,
                )
            if is_out:
                nc.sync.dma_start(
                    out=out_dram[l - (n_layers - 3)].rearrange(
                        "(m p) d2 -> p m d2", p=128
                    ),
                    in_=h_nat,
                )
            if l == n_layers - 1:
                nc.sync.dma_start(
                    out=out_dram[3].rearrange("(m p) d2 -> p m d2", p=128), in_=out3
                )

    nc.compile()
    return nc


def prep_inputs(g, h, W_down, b_down, W_bottom, b_bottom, W_up, b_up, c=C,
                nag=None):
    """Host-side sharding + layout prep. Returns per-core input maps."""
    if nag is None:
        nag = NAG_DEFAULT
    n = g.shape[0]
    s = n // c
    d = h.shape[1]
    g = np.asarray(g, np.float32)
    h = np.asarray(h, np.float32)
    deg = g.sum(axis=1) + 1.0
    dinv = (1.0 / np.sqrt(deg)).astype(np.float32)

    u0 = (h * dinv[:, None]).astype(np.float32)
    # part-major permutation: [part, rank, rows-within-part]
    u0_perm = np.ascontiguousarray(
        u0.reshape(c, nag, s // nag, d).transpose(1, 0, 2, 3).reshape(n, d)
    )
    u0_packed = np.asarray(u0_perm.astype(ml_bf16))

    Ws = [W_down[0], W_down[1], W_down[2], W_bottom, W_up[0], W_up[1], W_up[2]]
    bs = [b_down[0], b_down[1], b_down[2], b_bottom, b_up[0], b_up[1], b_up[2]]
    wt = np.stack(
        [np.ascontiguousarray(np.asarray(W, np.float32).T) for W in Ws]
    ).astype(ml_bf16)
    nl = len(Ws)
    bias_t = np.zeros((128, 2 * nl), np.float32)
    for li, b in enumerate(bs):
        b = np.asarray(b, np.float32)
        for dh in range(d // 128):
            bias_t[:, 2 * li + dh] = b[dh * 128 : (dh + 1) * 128]

    in_maps = []
    for ci in range(c):
        sl = slice(ci * s, (ci + 1) * s)
        a_slab = np.ascontiguousarray(g[:, sl])
        idx = np.arange(s)
        a_slab[ci * s + idx, idx] += 1.0  # fold self-loops into the slab
        dinv_slab = dinv[sl].reshape(s // 128, 128).T.copy()  # [128, mq]
        dinv_bcast = np.broadcast_to(dinv[sl][None, :], (128, s)).copy()
        in_maps.append(
            dict(
                a_slab=a_slab,
                u0=u0_packed,
                h0_slab=np.ascontiguousarray(h[sl]),
                dinv_slab=dinv_slab,
                dinv_bcast=dinv_bcast,
                wt=np.asarray(wt),
                bias_t=bias_t,
            )
        )
    return in_maps


try:
    import ml_dtypes

    ml_bf16 = ml_dtypes.bfloat16
except ImportError:  # pragma: no cover
    import jax.numpy as jnp

    ml_bf16 = jnp.bfloat16

_NC_CACHE = {}


def kernel(g, h, W_down, b_down, W_bottom, b_bottom, W_up, b_up):
    key = "full"
    if key not in _NC_CACHE:
        _NC_CACHE[key] = build_nc()
    nc = _NC_CACHE[key]
    in_maps = prep_inputs(g, h, W_down, b_down, W_bottom, b_bottom, W_up, b_up)
    res = run_bass_kernel_spmd(nc, in_maps, list(range(C)))
    outs = [np.asarray(r["out"]).reshape(4, S, D) for r in res.results]
    full = np.concatenate(outs, axis=1)  # [4, N, D]
    return full.astype(np.float32)


if __name__ == "__main__":
    import reference

    inputs = reference.setup_inputs()
    inputs = {k: np.asarray(v) for k, v in inputs.items()}
    out = kernel(**inputs)
    exp = np.asarray(reference.reference(**reference.setup_inputs()))
    err = np.abs(out - exp).max() / (np.abs(exp).max() + 1e-30)
    rel = np.linalg.norm(out - exp) / (np.linalg.norm(exp) + 1e-30)
    print("max-scaled err:", err, "rel l2:", rel)
